# revision 6
# baseline (speedup 1.0000x reference)
"""Trainium2 Bass kernel for the Plastic_RetroactiveModulated_RNN problem.

Math (per batch b):
    pre      = inputs @ i2h_w.T + i2h_b                          [HS]
    rec      = (w + alpha*pw[b]) @ hidden[b]                     [HS]
    hactiv   = tanh(pre + rec)
    heads    = [h2DA; h2o; h2v] @ hactiv + [h2DA_b; h2o_b; h2v_b]  (12 rows)
    DAout    = tanh(heads[0])
    et_new   = (1-etaet)*et + etaet * outer(hactiv, hidden)
    pw_new   = clip(pw + DAout*et, -1, 1)
    hebb_new = hebb  (pure passthrough -> returned host-side)

Distribution: pure data parallel over the batch dim, 16 batches on each of
the 8 NeuronCores; [HS,HS] params and head weights replicated.

Per-core layout: HS=500 is split into 4 chunks of 125 partitions. pw[b] and
et[b] stream through SBUF as single 1MB DMAs shaped [125, 4, 500]. The
shared part of rec (w@h + i2h@x + bias) is precomputed for all 16 batches
on the tensor engine using on-chip PE transposes; the per-batch modulated
part sum_j alpha[i,j]*pw[b,i,j]*h[b,j] is a DVE tensor_tensor_reduce of
pw against (alpha * broadcast(h)).
"""

import numpy as np
from contextlib import ExitStack

import concourse.bass as bass
import concourse.tile as tile
from concourse import mybir
from concourse.masks import make_identity

F32 = mybir.dt.float32
N_CORES = 8
B, IN, HS, OUT, NBDA = 128, 64, 500, 10, 1
NB = B // N_CORES  # batches per core
P = 125            # partitions per HS chunk
C = HS // P        # 4 chunks
NH = 12            # stacked head rows: [DA(1); out(10); v(1)]

_Alu = mybir.AluOpType
_Act = mybir.ActivationFunctionType


def _split_sync_waits(nc, limit=1):
    """The walrus build in this container encodes at most `limit` sync-wait
    per instruction; hoist excess waits onto preceding same-engine NoOps."""
    n_new = 0
    for f in nc.m.functions:
        for bb in f.blocks:
            out = []
            for inst in bb.instructions:
                si = inst.sync_info
                if si is not None and si.on_wait and len(si.on_wait) > limit:
                    waits = list(si.on_wait)
                    extra, keep = waits[:-limit], waits[-limit:]
                    for i in range(0, len(extra), limit):
                        nop = mybir.InstNoOp(
                            name=f"{inst.name}_ws{n_new}",
                            engine=inst.engine,
                            ins=[],
                            outs=[],
                            sync_info=mybir.SyncInfo(
                                on_wait=list(extra[i : i + limit]), on_update=[]
                            ),
                        )
                        out.append(nop)
                        n_new += 1
                    si.on_wait = keep
                out.append(inst)
            bb.instructions[:] = out
    return n_new


def _emit(nc, tc, ctx, t):
    const = ctx.enter_context(tc.tile_pool(name="const", bufs=1))
    stream = ctx.enter_context(tc.tile_pool(name="stream", bufs=2))
    scratch = ctx.enter_context(tc.tile_pool(name="scratch", bufs=3))
    small = ctx.enter_context(tc.tile_pool(name="small", bufs=3))
    ps_h = ctx.enter_context(tc.tile_pool(name="ps_h", bufs=2, space="PSUM"))
    ps_tr = ctx.enter_context(tc.tile_pool(name="ps_tr", bufs=2, space="PSUM"))
    ps_pr = ctx.enter_context(tc.tile_pool(name="ps_pr", bufs=2, space="PSUM"))
    ps_sm = ctx.enter_context(tc.tile_pool(name="ps_sm", bufs=2, space="PSUM"))

    ident = const.tile([128, 128], F32)
    make_identity(nc, ident)
    ones_row = const.tile([1, P], F32)
    nc.vector.memset(ones_row, 1.0)

    # ---- small parameter loads
    inputs_sb = const.tile([NB, IN], F32)
    nc.sync.dma_start(out=inputs_sb, in_=t["inputs"][:])
    hidden_sb = const.tile([NB, HS], F32)
    nc.sync.dma_start(out=hidden_sb, in_=t["hidden"][:])
    # same data as a single-partition row, so per-batch slices sit at base
    # partition 0 (matmul operands must start at partition 0/32/64)
    hidden_row = const.tile([1, NB * HS], F32)
    hflat = t["hidden"][:].rearrange("b j -> (b j)")
    nc.sync.dma_start(
        out=hidden_row,
        in_=bass.AP(tensor=hflat.tensor, offset=hflat.offset, ap=[[0, 1]] + list(hflat.ap)),
    )
    w_sb = const.tile([P, C, HS], F32)
    nc.sync.dma_start(out=w_sb, in_=t["w"][:].rearrange("(c p) j -> p c j", p=P))
    alpha_sb = const.tile([P, C, HS], F32)
    nc.sync.dma_start(out=alpha_sb, in_=t["alpha"][:].rearrange("(c p) j -> p c j", p=P))
    i2h_sb = const.tile([P, C, IN], F32)
    nc.sync.dma_start(out=i2h_sb, in_=t["i2h_w"][:].rearrange("(c p) k -> p c k", p=P))
    i2hb_sb = const.tile([P, C], F32)
    nc.sync.dma_start(out=i2hb_sb, in_=t["i2h_b"][:].rearrange("(c p) -> p c", p=P))
    heads_sb = const.tile([NH, HS], F32)
    nc.sync.dma_start(out=heads_sb[0:1, :], in_=t["h2DA_w"][:])
    nc.sync.dma_start(out=heads_sb[1 : 1 + OUT, :], in_=t["h2o_w"][:])
    nc.sync.dma_start(out=heads_sb[NH - 1 : NH, :], in_=t["h2v_w"][:])
    bias12_sb = const.tile([NH, 1], F32)
    nc.sync.dma_start(out=bias12_sb[0:1, :], in_=t["h2DA_b"][:])
    nc.sync.dma_start(out=bias12_sb[1 : 1 + OUT, :], in_=t["h2o_b"][:])
    nc.sync.dma_start(out=bias12_sb[NH - 1 : NH, :], in_=t["h2v_b"][:])
    etaet_sb = const.tile([1, 1], F32)
    nc.sync.dma_start(out=etaet_sb, in_=t["etaet"][:])

    # ---- etaet broadcast across partitions: etaet_bc, (1-etaet)_bc
    ps = ps_sm.tile([P, 1], F32, tag="sm")
    nc.tensor.matmul(ps, lhsT=ones_row, rhs=etaet_sb, start=True, stop=True)
    etaet_bc = const.tile([P, 1], F32)
    nc.scalar.copy(etaet_bc, ps)
    onem_bc = const.tile([P, 1], F32)
    nc.scalar.activation(onem_bc, ps, _Act.Copy, bias=1.0, scale=-1.0)

    # ---- PE transposes of the small operands
    pst = ps_tr.tile([IN, NB], F32, tag="tr")
    nc.tensor.transpose(pst, inputs_sb, ident[:NB, :NB])
    inT_sb = const.tile([IN, NB], F32)
    nc.scalar.copy(inT_sb, pst)

    hT_sb = const.tile([P, C, NB], F32)
    for cj in range(C):
        pst = ps_tr.tile([P, NB], F32, tag="tr")
        nc.tensor.transpose(pst, hidden_sb[:, cj * P : (cj + 1) * P], ident[:NB, :NB])
        nc.scalar.copy(hT_sb[:, cj, :], pst)

    wT_sb = const.tile([P, C, HS], F32)
    for ci in range(C):
        for cj in range(C):
            pst = ps_tr.tile([P, P], F32, tag="tr")
            nc.tensor.transpose(
                pst, w_sb[:, ci, cj * P : (cj + 1) * P], ident[:P, :P]
            )
            nc.scalar.copy(wT_sb[:, cj, ci * P : (ci + 1) * P], pst)

    i2hT_sb = const.tile([IN, HS], F32)
    for ci in range(C):
        pst = ps_tr.tile([IN, P], F32, tag="tr")
        nc.tensor.transpose(pst, i2h_sb[:, ci, :], ident[:P, :P])
        nc.scalar.copy(i2hT_sb[:, ci * P : (ci + 1) * P], pst)

    headsT_sb = const.tile([P, C, NH], F32)
    for ci in range(C):
        pst = ps_tr.tile([P, NH], F32, tag="tr")
        nc.tensor.transpose(pst, heads_sb[:, ci * P : (ci + 1) * P], ident[:NH, :NH])
        nc.scalar.copy(headsT_sb[:, ci, :], pst)

    # ---- shared rec part for all batches: pr[i, b] = (i2h@x + w@h + i2h_b)
    pr_sb = const.tile([P, C, NB], F32)
    for ci in range(C):
        psp = ps_pr.tile([P, NB], F32, tag="pr")
        nc.tensor.matmul(
            psp, lhsT=i2hT_sb[:, ci * P : (ci + 1) * P], rhs=inT_sb,
            start=True, stop=False,
        )
        for cj in range(C):
            nc.tensor.matmul(
                psp,
                lhsT=wT_sb[:, cj, ci * P : (ci + 1) * P],
                rhs=hT_sb[:, cj, :],
                start=False,
                stop=(cj == C - 1),
            )
        nc.scalar.activation(
            pr_sb[:, ci, :], psp, _Act.Identity, bias=i2hb_sb[:, ci : ci + 1], scale=1.0
        )

    hact_all = const.tile([P, C, NB], F32)
    heads_all = const.tile([NH, NB], F32)

    for b in range(NB):
        # broadcast hidden[b] to all partitions via PE (ones ⊗ h)
        psh = ps_h.tile([P, HS], F32, tag="h")
        nc.tensor.matmul(
            psh, lhsT=ones_row, rhs=hidden_row[:, b * HS : (b + 1) * HS],
            start=True, stop=True,
        )
        h_bc = scratch.tile([P, HS], F32, tag="h_bc")
        nc.scalar.copy(h_bc, psh)

        pw_sb = stream.tile([P, C, HS], F32, tag="pw_in")
        nc.sync.dma_start(out=pw_sb, in_=t["pw"][:][b].rearrange("(c p) j -> p c j", p=P))
        et_sb = stream.tile([P, C, HS], F32, tag="et_in")
        nc.sync.dma_start(out=et_sb, in_=t["et"][:][b].rearrange("(c p) j -> p c j", p=P))

        # rec2[i] = sum_j alpha[i,j]*pw[b,i,j]*h[b,j]; hact = tanh(rec2 + pr)
        rec = small.tile([P, C], F32, tag="rec")
        for ci in range(C):
            ah = scratch.tile([P, HS], F32, tag="ah")
            nc.vector.tensor_mul(ah, alpha_sb[:, ci, :], h_bc)
            m2 = scratch.tile([P, HS], F32, tag="m2")
            nc.vector.tensor_mul(m2, pw_sb[:, ci, :], ah)
            nc.vector.tensor_reduce(
                rec[:, ci : ci + 1], m2, axis=mybir.AxisListType.X, op=_Alu.add
            )
            nc.scalar.activation(
                hact_all[:, ci, b : b + 1],
                rec[:, ci : ci + 1],
                _Act.Tanh,
                bias=pr_sb[:, ci, b : b + 1],
                scale=1.0,
            )

        # heads = [DA; out; v] @ hact + bias  (PE partition reduction)
        ps12 = ps_sm.tile([NH, 1], F32, tag="sm")
        for ci in range(C):
            nc.tensor.matmul(
                ps12,
                lhsT=headsT_sb[:, ci, :],
                rhs=hact_all[:, ci, b : b + 1],
                start=(ci == 0),
                stop=(ci == C - 1),
            )
        nc.scalar.activation(
            heads_all[:, b : b + 1], ps12, _Act.Identity, bias=bias12_sb, scale=1.0
        )
        da1 = small.tile([1, 1], F32, tag="da1")
        nc.scalar.activation(da1, heads_all[0:1, b : b + 1], _Act.Tanh)
        psda = ps_sm.tile([P, 1], F32, tag="sm")
        nc.tensor.matmul(psda, lhsT=ones_row, rhs=da1, start=True, stop=True)
        da_bc = small.tile([P, 1], F32, tag="da_bc")
        nc.scalar.copy(da_bc, psda)

        # se[i] = etaet * hact[i]
        se = small.tile([P, C], F32, tag="se")
        nc.vector.tensor_scalar_mul(se, hact_all[:, :, b], etaet_bc)

        et_out = stream.tile([P, C, HS], F32, tag="et_out")
        pw_out = stream.tile([P, C, HS], F32, tag="pw_out")
        for ci in range(C):
            ets = scratch.tile([P, HS], F32, tag="ets")
            nc.scalar.activation(ets, et_sb[:, ci, :], _Act.Copy, scale=onem_bc)
            dh = scratch.tile([P, HS], F32, tag="dh")
            nc.scalar.activation(dh, h_bc, _Act.Copy, scale=se[:, ci : ci + 1])
            nc.gpsimd.tensor_add(et_out[:, ci, :], ets, dh)
            pwd = scratch.tile([P, HS], F32, tag="pwd")
            nc.scalar.activation(pwd, et_sb[:, ci, :], _Act.Copy, scale=da_bc)
            nc.gpsimd.tensor_add(pw_out[:, ci, :], pw_sb[:, ci, :], pwd)
            nc.vector.tensor_scalar(
                out=pw_out[:, ci, :],
                in0=pw_out[:, ci, :],
                scalar1=1.0,
                scalar2=-1.0,
                op0=_Alu.min,
                op1=_Alu.max,
            )
        nc.sync.dma_start(
            out=t["et_new"][:][b].rearrange("(c p) j -> p c j", p=P), in_=et_out
        )
        nc.sync.dma_start(
            out=t["pw_new"][:][b].rearrange("(c p) j -> p c j", p=P), in_=pw_out
        )

    # ---- outputs: hidden_new = hact (transpose back), activout/valueout
    hid_sb = const.tile([NB, HS], F32)
    for ci in range(C):
        pst = ps_tr.tile([NB, P], F32, tag="tr")
        nc.tensor.transpose(pst, hact_all[:, ci, :], ident[:P, :P])
        nc.scalar.copy(hid_sb[:, ci * P : (ci + 1) * P], pst)
    nc.sync.dma_start(out=t["hidden_new"][:], in_=hid_sb)

    pst = ps_tr.tile([NB, NH], F32, tag="tr")
    nc.tensor.transpose(pst, heads_all, ident[:NH, :NH])
    ho_sb = const.tile([NB, NH], F32)
    nc.scalar.copy(ho_sb, pst)
    nc.sync.dma_start(out=t["activout"][:], in_=ho_sb[:, 1 : 1 + OUT])
    nc.sync.dma_start(out=t["valueout"][:], in_=ho_sb[:, NH - 1 : NH])


_IN_SHAPES = {
    "inputs": [NB, IN],
    "hidden": [NB, HS],
    "et": [NB, HS, HS],
    "pw": [NB, HS, HS],
    "i2h_w": [HS, IN],
    "i2h_b": [HS],
    "w": [HS, HS],
    "alpha": [HS, HS],
    "etaet": [1, 1],
    "h2DA_w": [NBDA, HS],
    "h2DA_b": [NBDA, 1],
    "h2o_w": [OUT, HS],
    "h2o_b": [OUT, 1],
    "h2v_w": [1, HS],
    "h2v_b": [1, 1],
}
_OUT_SHAPES = {
    "activout": [NB, OUT],
    "valueout": [NB, 1],
    "hidden_new": [NB, HS],
    "et_new": [NB, HS, HS],
    "pw_new": [NB, HS, HS],
}
# host-side reshape of 1-D scalar/bias inputs to the declared 2-D shapes
_RESHAPE = {"etaet": (1, 1), "h2DA_b": (1, 1), "h2o_b": (OUT, 1), "h2v_b": (1, 1)}
_SHARDED_INPUTS = ("inputs", "hidden", "et", "pw")


def _build_bass():
    nc = bass.Bass()
    t = {}
    for name, shape in _IN_SHAPES.items():
        t[name] = nc.declare_dram_parameter(name, shape, F32, isOutput=False)
    for name, shape in _OUT_SHAPES.items():
        t[name] = nc.declare_dram_parameter(name, shape, F32, isOutput=True)
    with tile.TileContext(nc) as tc, ExitStack() as ctx:
        _emit(nc, tc, ctx, t)
    _split_sync_waits(nc)
    return nc


_RUNNER = None


def _get_runner():
    global _RUNNER
    if _RUNNER is not None:
        return _RUNNER
    import jax
    from jax.experimental.shard_map import shard_map
    from jax.sharding import Mesh, NamedSharding, PartitionSpec
    from concourse import bass2jax

    nc = _build_bass()
    bass2jax.install_neuronx_cc_hook()

    partition_name = (
        nc.partition_id_tensor.name if nc.partition_id_tensor is not None else None
    )
    in_names, out_names, out_avals = [], [], []
    for alloc in nc.m.functions[0].allocations:
        if not isinstance(alloc, mybir.MemoryLocationSet):
            continue
        name = alloc.memorylocations[0].name
        if alloc.kind == "ExternalInput":
            if name != partition_name:
                in_names.append(name)
        elif alloc.kind == "ExternalOutput":
            out_names.append(name)
            out_avals.append(
                jax.core.ShapedArray(tuple(alloc.tensor_shape), mybir.dt.np(alloc.dtype))
            )
    n_params = len(in_names)
    all_in_names = tuple(in_names) + tuple(out_names)
    if partition_name is not None:
        all_in_names = all_in_names + (partition_name,)

    def _body(*args):
        operands = list(args)
        if partition_name is not None:
            operands.append(bass2jax.partition_id_tensor())
        outs = bass2jax._bass_exec_p.bind(
            *operands,
            out_avals=tuple(out_avals),
            in_names=all_in_names,
            out_names=tuple(out_names),
            lowering_input_output_aliases=(),
            sim_require_finite=True,
            sim_require_nnan=True,
            nc=nc,
        )
        return tuple(outs)

    devices = jax.devices()[:N_CORES]
    mesh = Mesh(np.asarray(devices), ("core",))
    n_outs = len(out_names)
    sharded = jax.jit(
        shard_map(
            _body,
            mesh=mesh,
            in_specs=(PartitionSpec("core"),) * (n_params + n_outs),
            out_specs=(PartitionSpec("core"),) * n_outs,
            check_rep=False,
        ),
        keep_unused=True,
    )
    sharding = NamedSharding(mesh, PartitionSpec("core"))
    # outputs are fully written by the kernel, so the "zero" operand buffers
    # are never read: allocate them once on device and reuse (no donation)
    dev_zeros = [
        jax.device_put(
            np.zeros((N_CORES * av.shape[0], *av.shape[1:]), av.dtype), sharding
        )
        for av in out_avals
    ]
    _RUNNER = {
        "sharded": sharded,
        "in_names": in_names,
        "out_names": out_names,
        "dev_zeros": dev_zeros,
        "sharding": sharding,
        "mesh": mesh,
    }
    return _RUNNER


def _make_feed(np_inputs):
    """Build the global (concatenated-over-cores) operand list."""
    r = _get_runner()
    feed = []
    for name in r["in_names"]:
        arr = np.ascontiguousarray(np.asarray(np_inputs[name], dtype=np.float32))
        if name in _RESHAPE:
            arr = arr.reshape(_RESHAPE[name])
        if name in _SHARDED_INPUTS:
            feed.append(arr)  # [128, ...] shards naturally into 16/core
        else:
            feed.append(np.concatenate([arr] * N_CORES, axis=0))  # replicate
    return feed


def _run(feed):
    r = _get_runner()
    outs = r["sharded"](*feed, *r["dev_zeros"])
    return {name: outs[i] for i, name in enumerate(r["out_names"])}


def kernel(**inputs):
    feed = _make_feed(inputs)
    outs = _run(feed)
    activout = np.asarray(outs["activout"])
    valueout = np.asarray(outs["valueout"])
    hidden_new = np.asarray(outs["hidden_new"])
    et_new = np.asarray(outs["et_new"])
    pw_new = np.asarray(outs["pw_new"])
    hebb_new = np.asarray(inputs["hebb"], dtype=np.float32)  # untouched passthrough
    return activout, valueout, hidden_new, hebb_new, et_new, pw_new


# revision 14
# speedup vs baseline: 456.8121x; 456.8121x over previous
"""Trainium2 Bass kernel for the Plastic_RetroactiveModulated_RNN problem.

Math (per batch b):
    pre      = inputs @ i2h_w.T + i2h_b                          [HS]
    rec      = (w + alpha*pw[b]) @ hidden[b]                     [HS]
    hactiv   = tanh(pre + rec)
    heads    = [h2DA; h2o; h2v] @ hactiv + [h2DA_b; h2o_b; h2v_b]  (12 rows)
    DAout    = tanh(heads[0])
    et_new   = (1-etaet)*et + etaet * outer(hactiv, hidden)
    pw_new   = clip(pw + DAout*et, -1, 1)
    hebb_new = hebb  (pure passthrough -> returned host-side)

Distribution: pure data parallel over the batch dim, 16 batches on each of
the 8 NeuronCores; [HS,HS] params and head weights replicated.

Per-core layout: HS=500 is split into 4 chunks of 125 partitions. pw[b] and
et[b] stream through SBUF as single 1MB DMAs shaped [125, 4, 500]. The
shared part of rec (w@h + i2h@x + bias) is precomputed for all 16 batches
on the tensor engine using on-chip PE transposes; the per-batch modulated
part sum_j alpha[i,j]*pw[b,i,j]*h[b,j] is a DVE tensor_tensor_reduce of
pw against (alpha * broadcast(h)).
"""

import numpy as np
from contextlib import ExitStack

import concourse.bass as bass
import concourse.tile as tile
from concourse import mybir
from concourse.masks import make_identity

F32 = mybir.dt.float32
N_CORES = 8
B, IN, HS, OUT, NBDA = 128, 64, 500, 10, 1
NB = B // N_CORES  # batches per core
P = 125            # partitions per HS chunk
C = HS // P        # 4 chunks
NH = 12            # stacked head rows: [DA(1); out(10); v(1)]

_Alu = mybir.AluOpType
_Act = mybir.ActivationFunctionType


def _split_sync_waits(nc, limit=1):
    """The walrus build in this container encodes at most `limit` sync-wait
    per instruction; hoist excess waits onto preceding same-engine NoOps."""
    n_new = 0
    for f in nc.m.functions:
        for bb in f.blocks:
            out = []
            for inst in bb.instructions:
                si = inst.sync_info
                if si is not None and si.on_wait and len(si.on_wait) > limit:
                    waits = list(si.on_wait)
                    extra, keep = waits[:-limit], waits[-limit:]
                    for i in range(0, len(extra), limit):
                        nop = mybir.InstNoOp(
                            name=f"{inst.name}_ws{n_new}",
                            engine=inst.engine,
                            ins=[],
                            outs=[],
                            sync_info=mybir.SyncInfo(
                                on_wait=list(extra[i : i + limit]), on_update=[]
                            ),
                        )
                        out.append(nop)
                        n_new += 1
                    si.on_wait = keep
                out.append(inst)
            bb.instructions[:] = out
    return n_new


def _emit(nc, tc, ctx, t, rep=1):
    const = ctx.enter_context(tc.tile_pool(name="const", bufs=1))
    stream = ctx.enter_context(tc.tile_pool(name="stream", bufs=2))
    scratch = ctx.enter_context(tc.tile_pool(name="scratch", bufs=3))
    small = ctx.enter_context(tc.tile_pool(name="small", bufs=3))
    ps_h = ctx.enter_context(tc.tile_pool(name="ps_h", bufs=2, space="PSUM"))
    ps_tr = ctx.enter_context(tc.tile_pool(name="ps_tr", bufs=2, space="PSUM"))
    ps_pr = ctx.enter_context(tc.tile_pool(name="ps_pr", bufs=2, space="PSUM"))
    ps_sm = ctx.enter_context(tc.tile_pool(name="ps_sm", bufs=2, space="PSUM"))

    ident = const.tile([128, 128], F32)
    make_identity(nc, ident)
    ones_row = const.tile([1, P], F32)
    nc.vector.memset(ones_row, 1.0)

    # ---- small parameter loads
    inputs_sb = const.tile([NB, IN], F32)
    nc.sync.dma_start(out=inputs_sb, in_=t["inputs"][:])
    hidden_sb = const.tile([NB, HS], F32)
    nc.sync.dma_start(out=hidden_sb, in_=t["hidden"][:])
    # same data as a single-partition row, so per-batch slices sit at base
    # partition 0 (matmul operands must start at partition 0/32/64)
    hidden_row = const.tile([1, NB * HS], F32)
    hflat = t["hidden"][:].rearrange("b j -> (b j)")
    nc.sync.dma_start(
        out=hidden_row,
        in_=bass.AP(tensor=hflat.tensor, offset=hflat.offset, ap=[[0, 1]] + list(hflat.ap)),
    )
    w_sb = const.tile([P, C, HS], F32)
    nc.sync.dma_start(out=w_sb, in_=t["w"][:].rearrange("(c p) j -> p c j", p=P))
    alpha_sb = const.tile([P, C, HS], F32)
    nc.sync.dma_start(out=alpha_sb, in_=t["alpha"][:].rearrange("(c p) j -> p c j", p=P))
    i2h_sb = const.tile([P, C, IN], F32)
    nc.sync.dma_start(out=i2h_sb, in_=t["i2h_w"][:].rearrange("(c p) k -> p c k", p=P))
    i2hb_sb = const.tile([P, C], F32)
    nc.sync.dma_start(out=i2hb_sb, in_=t["i2h_b"][:].rearrange("(c p) -> p c", p=P))
    heads_sb = const.tile([NH, HS], F32)
    nc.sync.dma_start(out=heads_sb[0:1, :], in_=t["h2DA_w"][:])
    nc.sync.dma_start(out=heads_sb[1 : 1 + OUT, :], in_=t["h2o_w"][:])
    nc.sync.dma_start(out=heads_sb[NH - 1 : NH, :], in_=t["h2v_w"][:])
    bias12_sb = const.tile([NH, 1], F32)
    nc.sync.dma_start(out=bias12_sb[0:1, :], in_=t["h2DA_b"][:])
    nc.sync.dma_start(out=bias12_sb[1 : 1 + OUT, :], in_=t["h2o_b"][:])
    nc.sync.dma_start(out=bias12_sb[NH - 1 : NH, :], in_=t["h2v_b"][:])
    etaet_sb = const.tile([1, 1], F32)
    nc.sync.dma_start(out=etaet_sb, in_=t["etaet"][:])

    # ---- etaet broadcast across partitions: etaet_bc, (1-etaet)_bc
    ps = ps_sm.tile([P, 1], F32, tag="sm")
    nc.tensor.matmul(ps, lhsT=ones_row, rhs=etaet_sb, start=True, stop=True)
    etaet_bc = const.tile([P, 1], F32)
    nc.scalar.copy(etaet_bc, ps)
    onem_bc = const.tile([P, 1], F32)
    nc.scalar.activation(onem_bc, ps, _Act.Copy, bias=1.0, scale=-1.0)

    # ---- PE transposes of the small operands
    pst = ps_tr.tile([IN, NB], F32, tag="tr")
    nc.tensor.transpose(pst, inputs_sb, ident[:NB, :NB])
    inT_sb = const.tile([IN, NB], F32)
    nc.scalar.copy(inT_sb, pst)

    hT_sb = const.tile([P, C, NB], F32)
    for cj in range(C):
        pst = ps_tr.tile([P, NB], F32, tag="tr")
        nc.tensor.transpose(pst, hidden_sb[:, cj * P : (cj + 1) * P], ident[:NB, :NB])
        nc.scalar.copy(hT_sb[:, cj, :], pst)

    wT_sb = const.tile([P, C, HS], F32)
    for ci in range(C):
        for cj in range(C):
            pst = ps_tr.tile([P, P], F32, tag="tr")
            nc.tensor.transpose(
                pst, w_sb[:, ci, cj * P : (cj + 1) * P], ident[:P, :P]
            )
            nc.scalar.copy(wT_sb[:, cj, ci * P : (ci + 1) * P], pst)

    i2hT_sb = const.tile([IN, HS], F32)
    for ci in range(C):
        pst = ps_tr.tile([IN, P], F32, tag="tr")
        nc.tensor.transpose(pst, i2h_sb[:, ci, :], ident[:P, :P])
        nc.scalar.copy(i2hT_sb[:, ci * P : (ci + 1) * P], pst)

    headsT_sb = const.tile([P, C, NH], F32)
    for ci in range(C):
        pst = ps_tr.tile([P, NH], F32, tag="tr")
        nc.tensor.transpose(pst, heads_sb[:, ci * P : (ci + 1) * P], ident[:NH, :NH])
        nc.scalar.copy(headsT_sb[:, ci, :], pst)

    # ---- shared rec part for all batches: pr[i, b] = (i2h@x + w@h + i2h_b)
    pr_sb = const.tile([P, C, NB], F32)
    for ci in range(C):
        psp = ps_pr.tile([P, NB], F32, tag="pr")
        nc.tensor.matmul(
            psp, lhsT=i2hT_sb[:, ci * P : (ci + 1) * P], rhs=inT_sb,
            start=True, stop=False,
        )
        for cj in range(C):
            nc.tensor.matmul(
                psp,
                lhsT=wT_sb[:, cj, ci * P : (ci + 1) * P],
                rhs=hT_sb[:, cj, :],
                start=False,
                stop=(cj == C - 1),
            )
        nc.scalar.activation(
            pr_sb[:, ci, :], psp, _Act.Identity, bias=i2hb_sb[:, ci : ci + 1], scale=1.0
        )

    hact_all = const.tile([P, C, NB], F32)
    heads_all = const.tile([NH, NB], F32)

    # rep>1 repeats the batch loop inside one NEFF (identical writes) so a
    # rep-slope wall-clock measurement isolates HW time from dispatch cost
    for b in [b for _ in range(rep) for b in range(NB)]:
        # broadcast hidden[b] to all partitions via PE (ones ⊗ h)
        psh = ps_h.tile([P, HS], F32, tag="h")
        nc.tensor.matmul(
            psh, lhsT=ones_row, rhs=hidden_row[:, b * HS : (b + 1) * HS],
            start=True, stop=True,
        )
        h_bc = scratch.tile([P, HS], F32, tag="h_bc")
        nc.scalar.copy(h_bc, psh)

        pw_sb = stream.tile([P, C, HS], F32, tag="pw_in")
        nc.sync.dma_start(out=pw_sb, in_=t["pw"][:][b].rearrange("(c p) j -> p c j", p=P))
        et_sb = stream.tile([P, C, HS], F32, tag="et_in")
        nc.sync.dma_start(out=et_sb, in_=t["et"][:][b].rearrange("(c p) j -> p c j", p=P))

        # rec2[i] = sum_j alpha[i,j]*pw[b,i,j]*h[b,j]; hact = tanh(rec2 + pr)
        rec = small.tile([P, C], F32, tag="rec")
        for ci in range(C):
            ah = scratch.tile([P, HS], F32, tag="ah")
            nc.vector.tensor_mul(ah, alpha_sb[:, ci, :], h_bc)
            m2 = scratch.tile([P, HS], F32, tag="m2")
            nc.vector.tensor_mul(m2, pw_sb[:, ci, :], ah)
            nc.vector.tensor_reduce(
                rec[:, ci : ci + 1], m2, axis=mybir.AxisListType.X, op=_Alu.add
            )
            nc.scalar.activation(
                hact_all[:, ci, b : b + 1],
                rec[:, ci : ci + 1],
                _Act.Tanh,
                bias=pr_sb[:, ci, b : b + 1],
                scale=1.0,
            )

        # heads = [DA; out; v] @ hact + bias  (PE partition reduction)
        ps12 = ps_sm.tile([NH, 1], F32, tag="sm")
        for ci in range(C):
            nc.tensor.matmul(
                ps12,
                lhsT=headsT_sb[:, ci, :],
                rhs=hact_all[:, ci, b : b + 1],
                start=(ci == 0),
                stop=(ci == C - 1),
            )
        nc.scalar.activation(
            heads_all[:, b : b + 1], ps12, _Act.Identity, bias=bias12_sb, scale=1.0
        )
        da1 = small.tile([1, 1], F32, tag="da1")
        nc.scalar.activation(da1, heads_all[0:1, b : b + 1], _Act.Tanh)
        psda = ps_sm.tile([P, 1], F32, tag="sm")
        nc.tensor.matmul(psda, lhsT=ones_row, rhs=da1, start=True, stop=True)
        da_bc = small.tile([P, 1], F32, tag="da_bc")
        nc.scalar.copy(da_bc, psda)

        # se[i] = etaet * hact[i]
        se = small.tile([P, C], F32, tag="se")
        nc.vector.tensor_scalar_mul(se, hact_all[:, :, b], etaet_bc)

        et_out = stream.tile([P, C, HS], F32, tag="et_out")
        pw_out = stream.tile([P, C, HS], F32, tag="pw_out")
        for ci in range(C):
            ets = scratch.tile([P, HS], F32, tag="ets")
            nc.scalar.activation(ets, et_sb[:, ci, :], _Act.Copy, scale=onem_bc)
            dh = scratch.tile([P, HS], F32, tag="dh")
            nc.scalar.activation(dh, h_bc, _Act.Copy, scale=se[:, ci : ci + 1])
            nc.gpsimd.tensor_add(et_out[:, ci, :], ets, dh)
            pwd = scratch.tile([P, HS], F32, tag="pwd")
            nc.scalar.activation(pwd, et_sb[:, ci, :], _Act.Copy, scale=da_bc)
            nc.gpsimd.tensor_add(pw_out[:, ci, :], pw_sb[:, ci, :], pwd)
            nc.vector.tensor_scalar(
                out=pw_out[:, ci, :],
                in0=pw_out[:, ci, :],
                scalar1=1.0,
                scalar2=-1.0,
                op0=_Alu.min,
                op1=_Alu.max,
            )
        nc.sync.dma_start(
            out=t["et_new"][:][b].rearrange("(c p) j -> p c j", p=P), in_=et_out
        )
        nc.sync.dma_start(
            out=t["pw_new"][:][b].rearrange("(c p) j -> p c j", p=P), in_=pw_out
        )

    # ---- outputs: hidden_new = hact (transpose back), activout/valueout
    hid_sb = const.tile([NB, HS], F32)
    for ci in range(C):
        pst = ps_tr.tile([NB, P], F32, tag="tr")
        nc.tensor.transpose(pst, hact_all[:, ci, :], ident[:P, :P])
        nc.scalar.copy(hid_sb[:, ci * P : (ci + 1) * P], pst)
    nc.sync.dma_start(out=t["hidden_new"][:], in_=hid_sb)

    pst = ps_tr.tile([NB, NH], F32, tag="tr")
    nc.tensor.transpose(pst, heads_all, ident[:NH, :NH])
    ho_sb = const.tile([NB, NH], F32)
    nc.scalar.copy(ho_sb, pst)
    nc.sync.dma_start(out=t["activout"][:], in_=ho_sb[:, 1 : 1 + OUT])
    nc.sync.dma_start(out=t["valueout"][:], in_=ho_sb[:, NH - 1 : NH])


_IN_SHAPES = {
    "inputs": [NB, IN],
    "hidden": [NB, HS],
    "et": [NB, HS, HS],
    "pw": [NB, HS, HS],
    "i2h_w": [HS, IN],
    "i2h_b": [HS],
    "w": [HS, HS],
    "alpha": [HS, HS],
    "etaet": [1, 1],
    "h2DA_w": [NBDA, HS],
    "h2DA_b": [NBDA, 1],
    "h2o_w": [OUT, HS],
    "h2o_b": [OUT, 1],
    "h2v_w": [1, HS],
    "h2v_b": [1, 1],
}
_OUT_SHAPES = {
    "activout": [NB, OUT],
    "valueout": [NB, 1],
    "hidden_new": [NB, HS],
    "et_new": [NB, HS, HS],
    "pw_new": [NB, HS, HS],
}
# host-side reshape of 1-D scalar/bias inputs to the declared 2-D shapes
_RESHAPE = {"etaet": (1, 1), "h2DA_b": (1, 1), "h2o_b": (OUT, 1), "h2v_b": (1, 1)}
_SHARDED_INPUTS = ("inputs", "hidden", "et", "pw")


def _build_bass(rep=1):
    nc = bass.Bass()
    t = {}
    for name, shape in _IN_SHAPES.items():
        t[name] = nc.declare_dram_parameter(name, shape, F32, isOutput=False)
    for name, shape in _OUT_SHAPES.items():
        t[name] = nc.declare_dram_parameter(name, shape, F32, isOutput=True)
    with tile.TileContext(nc) as tc, ExitStack() as ctx:
        _emit(nc, tc, ctx, t, rep=rep)
    _split_sync_waits(nc)
    return nc


_RUNNERS = {}


def _get_runner(rep=1):
    if rep in _RUNNERS:
        return _RUNNERS[rep]
    import jax
    from jax.experimental.shard_map import shard_map
    from jax.sharding import Mesh, NamedSharding, PartitionSpec
    from concourse import bass2jax

    nc = _build_bass(rep=rep)
    bass2jax.install_neuronx_cc_hook()

    partition_name = (
        nc.partition_id_tensor.name if nc.partition_id_tensor is not None else None
    )
    in_names, out_names, out_avals = [], [], []
    for alloc in nc.m.functions[0].allocations:
        if not isinstance(alloc, mybir.MemoryLocationSet):
            continue
        name = alloc.memorylocations[0].name
        if alloc.kind == "ExternalInput":
            if name != partition_name:
                in_names.append(name)
        elif alloc.kind == "ExternalOutput":
            out_names.append(name)
            out_avals.append(
                jax.core.ShapedArray(tuple(alloc.tensor_shape), mybir.dt.np(alloc.dtype))
            )
    n_params = len(in_names)
    all_in_names = tuple(in_names) + tuple(out_names)
    if partition_name is not None:
        all_in_names = all_in_names + (partition_name,)

    def _body(*args):
        operands = list(args)
        if partition_name is not None:
            operands.append(bass2jax.partition_id_tensor())
        outs = bass2jax._bass_exec_p.bind(
            *operands,
            out_avals=tuple(out_avals),
            in_names=all_in_names,
            out_names=tuple(out_names),
            lowering_input_output_aliases=(),
            sim_require_finite=True,
            sim_require_nnan=True,
            nc=nc,
        )
        return tuple(outs)

    devices = jax.devices()[:N_CORES]
    mesh = Mesh(np.asarray(devices), ("core",))
    n_outs = len(out_names)
    sharded = jax.jit(
        shard_map(
            _body,
            mesh=mesh,
            in_specs=(PartitionSpec("core"),) * (n_params + n_outs),
            out_specs=(PartitionSpec("core"),) * n_outs,
            check_rep=False,
        ),
        keep_unused=True,
    )
    sharding = NamedSharding(mesh, PartitionSpec("core"))
    # outputs are fully written by the kernel, so the "zero" operand buffers
    # are never read: allocate them once on device and reuse (no donation)
    dev_zeros = [
        jax.device_put(
            np.zeros((N_CORES * av.shape[0], *av.shape[1:]), av.dtype), sharding
        )
        for av in out_avals
    ]
    _RUNNERS[rep] = {
        "sharded": sharded,
        "in_names": in_names,
        "out_names": out_names,
        "dev_zeros": dev_zeros,
        "sharding": sharding,
        "mesh": mesh,
        "nc": nc,
    }
    return _RUNNERS[rep]


def _make_feed(np_inputs):
    """Build the global (concatenated-over-cores) operand list."""
    r = _get_runner()
    feed = []
    for name in r["in_names"]:
        arr = np.ascontiguousarray(np.asarray(np_inputs[name], dtype=np.float32))
        if name in _RESHAPE:
            arr = arr.reshape(_RESHAPE[name])
        if name in _SHARDED_INPUTS:
            feed.append(arr)  # [128, ...] shards naturally into 16/core
        else:
            feed.append(np.concatenate([arr] * N_CORES, axis=0))  # replicate
    return feed


def _run(feed):
    r = _get_runner()
    outs = r["sharded"](*feed, *r["dev_zeros"])
    return {name: outs[i] for i, name in enumerate(r["out_names"])}


def kernel(**inputs):
    feed = _make_feed(inputs)
    outs = _run(feed)
    activout = np.asarray(outs["activout"])
    valueout = np.asarray(outs["valueout"])
    hidden_new = np.asarray(outs["hidden_new"])
    et_new = np.asarray(outs["et_new"])
    pw_new = np.asarray(outs["pw_new"])
    hebb_new = np.asarray(inputs["hebb"], dtype=np.float32)  # untouched passthrough
    return activout, valueout, hidden_new, hebb_new, et_new, pw_new
